# revision 1
# baseline (speedup 1.0000x reference)
"""HGT link predictor on 8 Trainium2 NeuronCores (Bass/Tile SPMD kernel).

Strategy (hardcoded for nn_HGTLinkPredictor, N=50000 E=800000 P=100000 C=128 H=4 D=32):
 - Shard dst nodes (and their incoming edges) across 8 cores in contiguous
   128-node blocks. Edges sorted by dst on host.
 - Per layer: each core computes q/k/v projections for its node shard
   (relation transforms + attention scaling folded into the weights on host),
   AllGathers k/v so every core can gather k[src], v[src] rows by indirect DMA.
 - Segment softmax/weighted-sum over incoming edges of each dst node are done
   per 128-node block with one-hot selection matrices multiplied on the PE
   into PSUM accumulators (denominator division is algebraically deferred to
   the block epilogue, so a single pass over edges suffices).
 - Link decode is data-parallel over candidate edges with indirect gathers of
   the final node embeddings (AllGathered once).
"""

import math
import numpy as np
from contextlib import ExitStack

import concourse.bass as bass
import concourse.tile as tile
from concourse import bacc, mybir
from concourse import bass_utils
from concourse.masks import make_identity

F32 = mybir.dt.float32
I32 = mybir.dt.int32
AF = mybir.ActivationFunctionType
OP = mybir.AluOpType

CORES = 8
EPS = 1e-30


def _expand_last(ap, n):
    """Append a step-0 (broadcast) innermost dim of size n to an AP."""
    new = [list(p) for p in ap.ap] + [[0, n]]
    return bass.AP(ap.tensor, ap.offset, new)


# ----------------------------------------------------------------- host prep

def _host_prep(x, edge_index, pos_edge_index, neg_edge_index):
    N, C = x.shape
    E = edge_index.shape[1]
    P = pos_edge_index.shape[1]

    NPC = int(math.ceil(N / (CORES * 128))) * 128   # nodes per core (padded)
    BPC = NPC // 128                                # blocks per core
    NPAD = NPC * CORES

    src = edge_index[0].astype(np.int64)
    dst = edge_index[1].astype(np.int64)
    order = np.argsort(dst, kind="stable")
    s_src, s_dst = src[order], dst[order]

    core_of = s_dst // NPC
    blk_of = (s_dst % NPC) // 128
    gblk = core_of * BPC + blk_of          # global block id 0..CORES*BPC-1

    # per (core, block) counts -> uniform tile counts per block index
    cnt = np.zeros((CORES, BPC), dtype=np.int64)
    np.add.at(cnt, (core_of, blk_of), 1)
    T_b = np.maximum(1, np.ceil(cnt.max(axis=0) / 128).astype(np.int64))  # [BPC]
    tiles_total = int(T_b.sum())

    # slot in edge arrays for each sorted edge: per (core, block) sequential
    blk_starts = np.concatenate([[0], np.cumsum(T_b)])[:-1] * 128  # per block idx within core
    # position of edge within its (core, block) group
    # edges are sorted by dst hence grouped by gblk already
    grp_start = np.zeros(CORES * BPC + 1, dtype=np.int64)
    np.add.at(grp_start, gblk + 1, 1)
    grp_start = np.cumsum(grp_start)
    pos_in_grp = np.arange(E) - grp_start[gblk]

    cap = tiles_total * 128
    ekv = np.zeros((CORES, cap), dtype=np.int32)     # gather row in kv_full
    eqr = np.zeros((CORES, cap), dtype=np.int32)     # gather row in q_dram (local)
    eslot = np.full((CORES, cap), -1.0, dtype=np.float32)  # -1 => padding edge

    flat_pos = blk_starts[blk_of] + pos_in_grp       # position within core's edge array
    r = s_src // NPC
    i = s_src % NPC
    kv_row = r * (2 * NPC) + i
    np_c = core_of.astype(np.int64)
    ekv[np_c, flat_pos] = kv_row.astype(np.int32)
    eqr[np_c, flat_pos] = (s_dst - np_c * NPC).astype(np.int32)
    eslot[np_c, flat_pos] = (s_dst % 128).astype(np.float32)

    # reshape to [128, tiles_total] partition-major: entry [p, t] = edge t*128+p
    ekv = ekv.reshape(CORES, tiles_total, 128).transpose(0, 2, 1).copy()
    eqr = eqr.reshape(CORES, tiles_total, 128).transpose(0, 2, 1).copy()
    eslot = eslot.reshape(CORES, tiles_total, 128).transpose(0, 2, 1).copy()

    # decode shards
    PC = int(math.ceil(P / CORES))
    DT = int(math.ceil(PC / 128))
    PPC = DT * 128
    dec = np.zeros((CORES, 128, 4 * DT), dtype=np.int32)
    valid = []
    for c in range(CORES):
        lo = min(c * PC, P)
        hi = min(lo + PC, P)
        valid.append(hi - lo)
        for g, arr in enumerate((pos_edge_index[0], pos_edge_index[1],
                                 neg_edge_index[0], neg_edge_index[1])):
            a = np.zeros(PPC, dtype=np.int32)
            a[: hi - lo] = arr[lo:hi]
            dec[c, :, g * DT:(g + 1) * DT] = a.reshape(DT, 128).T

    # x shards (zero-padded)
    xs = np.zeros((CORES, NPC, C), dtype=np.float32)
    xpad = np.zeros((NPAD, C), dtype=np.float32)
    xpad[:N] = x
    for c in range(CORES):
        xs[c] = xpad[c * NPC:(c + 1) * NPC]

    meta = dict(N=N, C=C, E=E, P=P, NPC=NPC, BPC=BPC, NPAD=NPAD,
                T_b=tuple(int(t) for t in T_b), tiles_total=tiles_total,
                DT=DT, PC=PC, valid=valid)
    arrays = dict(ekv=ekv, eqr=eqr, eslot=eslot, dec=dec, xs=xs)
    return meta, arrays


def _prep_weights(inputs, H, D):
    """Fold relation transforms + attention scale into the linear weights."""
    C = inputs["W1k"].shape[0]
    out = {}
    for l in (1, 2):
        a_rel = np.asarray(inputs[f"a{l}"], np.float64)   # [H,D,D] (k transform)
        m_rel = np.asarray(inputs[f"m{l}"], np.float64)   # [H,D,D] (v transform)
        p_rel = np.asarray(inputs[f"p{l}"], np.float64)   # [H]
        A = np.zeros((C, C)); M = np.zeros((C, C))
        for h in range(H):
            A[h * D:(h + 1) * D, h * D:(h + 1) * D] = a_rel[h]
            M[h * D:(h + 1) * D, h * D:(h + 1) * D] = m_rel[h]
        qscale = np.repeat(p_rel / np.sqrt(D), D)         # [C]
        Wq = np.asarray(inputs[f"W{l}q"], np.float64) * qscale
        bq = np.asarray(inputs[f"b{l}q"], np.float64) * qscale
        Wk = np.asarray(inputs[f"W{l}k"], np.float64) @ A
        bk = np.asarray(inputs[f"b{l}k"], np.float64) @ A
        Wv = np.asarray(inputs[f"W{l}v"], np.float64) @ M
        bv = np.asarray(inputs[f"b{l}v"], np.float64) @ M
        a_sig = float(1.0 / (1.0 + np.exp(-float(inputs[f"skip{l}"]))))
        out[f"Wq{l}"] = Wq.astype(np.float32)
        out[f"Wk{l}"] = Wk.astype(np.float32)
        out[f"Wv{l}"] = Wv.astype(np.float32)
        out[f"Wo{l}"] = np.asarray(inputs[f"Wo{l}"], np.float32)
        out[f"bq{l}"] = np.broadcast_to(bq.astype(np.float32), (128, C)).copy()
        out[f"bk{l}"] = np.broadcast_to(bk.astype(np.float32), (128, C)).copy()
        out[f"bv{l}"] = np.broadcast_to(bv.astype(np.float32), (128, C)).copy()
        out[f"boa{l}"] = np.broadcast_to(
            (a_sig * np.asarray(inputs[f"bo{l}"], np.float64)).astype(np.float32),
            (128, C)).copy()
        out[f"asig{l}"] = a_sig
    Wlp = np.asarray(inputs["Wlp"], np.float32)
    out["w1"] = np.broadcast_to(Wlp[:C, 0], (128, C)).copy()
    out["w2"] = np.broadcast_to(Wlp[C:, 0], (128, C)).copy()
    out["blp"] = float(np.asarray(inputs["blp"]).reshape(-1)[0])
    return out


# ------------------------------------------------------------------- program

def _build_program(meta, asig1, asig2, blp, gelu_mode="hw"):
    NPC, BPC, NPAD = meta["NPC"], meta["BPC"], meta["NPAD"]
    T_b, tiles_total, DT = meta["T_b"], meta["tiles_total"], meta["DT"]
    Tmax = max(T_b)
    C = meta["C"]

    nc = bacc.Bacc("TRN2", target_bir_lowering=False, debug=False,
                   num_devices=CORES)

    # --- I/O -------------------------------------------------------------
    x_in = nc.dram_tensor("x_shard", [NPC, C], F32, kind="ExternalInput").ap()
    ekv_in = nc.dram_tensor("ekv", [128, tiles_total], I32, kind="ExternalInput").ap()
    eqr_in = nc.dram_tensor("eqr", [128, tiles_total], I32, kind="ExternalInput").ap()
    eslot_in = nc.dram_tensor("eslot", [128, tiles_total], F32, kind="ExternalInput").ap()
    wnames = []
    for l in (1, 2):
        wnames += [f"Wq{l}", f"Wk{l}", f"Wv{l}", f"Wo{l}",
                   f"bq{l}", f"bk{l}", f"bv{l}", f"boa{l}"]
    wnames += ["w1", "w2"]
    w_in = {n: nc.dram_tensor(n, [128, C], F32, kind="ExternalInput").ap()
            for n in wnames}
    uv_out = nc.dram_tensor("uv_out", [NPC, 2], F32, kind="ExternalOutput").ap()

    with tile.TileContext(nc) as tc, ExitStack() as ctx:
        sb = ctx.enter_context(tc.tile_pool(name="sb", bufs=2))
        cpool = ctx.enter_context(tc.tile_pool(name="const", bufs=1))
        psum = ctx.enter_context(tc.tile_pool(name="ps", bufs=2, space="PSUM"))
        dram = ctx.enter_context(tc.tile_pool(name="dr", bufs=1, space="DRAM"))

        # --- constants into SBUF ----------------------------------------
        W = {}
        for n in wnames:
            W[n] = cpool.tile([128, C], F32, tag=f"w_{n}", name=f"wt_{n}")
            nc.sync.dma_start(W[n][:], w_in[n][:])
        ekv_sb = cpool.tile([128, tiles_total], I32, tag="ekv")
        nc.sync.dma_start(ekv_sb[:], ekv_in[:])
        eqr_sb = cpool.tile([128, tiles_total], I32, tag="eqr")
        nc.sync.dma_start(eqr_sb[:], eqr_in[:])
        eslot_sb = cpool.tile([128, tiles_total], F32, tag="eslot")
        nc.sync.dma_start(eslot_sb[:], eslot_in[:])

        ident = cpool.tile([128, 128], F32, tag="ident")
        make_identity(nc, ident[:])
        iota_i = cpool.tile([128, Tmax * 128], I32, tag="iota_i")
        nc.gpsimd.iota(iota_i[:], pattern=[[0, Tmax], [1, 128]], base=0,
                       channel_multiplier=0)
        iota_f = cpool.tile([128, Tmax * 128], F32, tag="iota_f")
        nc.vector.tensor_copy(iota_f[:], iota_i[:])

        # --- DRAM scratch ------------------------------------------------
        q_dram = [dram.tile([NPC, C], F32, tag=f"q{l}", name=f"q_dram{l}") for l in (0, 1)]
        kv_shard = [dram.tile([2 * NPC, C], F32, tag=f"kvs{l}", name=f"kv_shard{l}") for l in (0, 1)]
        kv_full = [dram.tile([CORES * 2 * NPC, C], F32, tag=f"kvf{l}", name=f"kv_full{l}") for l in (0, 1)]
        h1_dram = dram.tile([NPC, C], F32, tag="h1")
        z_shard = dram.tile([NPC, C], F32, tag="zs")

        def layer(li, src_feat, h_out, asig):
            l = li + 1
            qd, kvs, kvf = q_dram[li], kv_shard[li], kv_full[li]
            # ---- projections for own shard ----
            for i in range(BPC):
                f = sb.tile([128, C], F32, tag="fblk")
                nc.sync.dma_start(f[:], src_feat[i * 128:(i + 1) * 128, :])
                fT_ps = psum.tile([128, 128], F32, tag="tr")
                nc.tensor.transpose(out=fT_ps[:], in_=f[:], identity=ident[:])
                fT = sb.tile([128, 128], F32, tag="fT")
                nc.vector.tensor_copy(fT[:], fT_ps[:])
                for wn, bn, dst, roff in ((f"Wq{l}", f"bq{l}", qd, 0),
                                          (f"Wk{l}", f"bk{l}", kvs, 0),
                                          (f"Wv{l}", f"bv{l}", kvs, NPC)):
                    ps = psum.tile([128, 128], F32, tag="mm")
                    nc.tensor.matmul(out=ps[:], lhsT=fT[:], rhs=W[wn][:],
                                     start=True, stop=True)
                    o = sb.tile([128, C], F32, tag="proj_o")
                    nc.vector.tensor_tensor(out=o[:], in0=ps[:], in1=W[bn][:],
                                            op=OP.add)
                    nc.sync.dma_start(
                        dst[roff + i * 128: roff + (i + 1) * 128, :], o[:])
            # ---- exchange k/v ----
            nc.gpsimd.collective_compute(
                "AllGather", OP.bypass,
                replica_groups=[list(range(CORES))],
                ins=[kvs[:]], outs=[kvf[:]])
            # ---- edge phase ----
            col = 0
            for b in range(BPC):
                T = T_b[b]
                Wd = T * 128
                kg = sb.tile([128, Tmax * 128], F32, tag="kg")
                vg = sb.tile([128, Tmax * 128], F32, tag="vg")
                qg = sb.tile([128, Tmax * 128], F32, tag="qg")
                for t in range(T):
                    sl = slice(t * 128, (t + 1) * 128)
                    nc.gpsimd.indirect_dma_start(
                        out=kg[:, sl], out_offset=None, in_=kvf[:],
                        in_offset=bass.IndirectOffsetOnAxis(
                            ap=ekv_sb[:, col + t:col + t + 1], axis=0))
                    nc.gpsimd.indirect_dma_start(
                        out=vg[:, sl], out_offset=None, in_=kvf[:],
                        in_offset=bass.IndirectOffsetOnAxis(
                            ap=ekv_sb[:, col + t:col + t + 1], axis=0),
                        element_offset=NPC * C)
                    nc.gpsimd.indirect_dma_start(
                        out=qg[:, sl], out_offset=None, in_=qd[:],
                        in_offset=bass.IndirectOffsetOnAxis(
                            ap=eqr_sb[:, col + t:col + t + 1], axis=0))
                S = sb.tile([128, Tmax * 128], F32, tag="S")
                nc.vector.tensor_tensor(
                    out=S[:, :Wd].rearrange("p (t j) -> p t j", j=128),
                    in0=iota_f[:, :Wd].rearrange("p (t j) -> p t j", j=128),
                    in1=_expand_last(eslot_sb[:, col:col + T], 128),
                    op=OP.is_equal)
                # qk dot per head
                nc.vector.tensor_tensor(out=kg[:, :Wd], in0=kg[:, :Wd],
                                        in1=qg[:, :Wd], op=OP.mult)
                alpha = sb.tile([128, Tmax * 4], F32, tag="alpha")
                nc.vector.tensor_reduce(
                    out=alpha[:, :T * 4],
                    in_=kg[:, :Wd].rearrange("p (x d) -> p x d", d=32),
                    axis=mybir.AxisListType.X, op=OP.add)
                ex = sb.tile([128, Tmax * 4], F32, tag="ex")
                nc.scalar.activation(ex[:, :T * 4], alpha[:, :T * 4], AF.Exp)
                # u = v * ex (broadcast over D)
                nc.vector.tensor_tensor(
                    out=vg[:, :Wd].rearrange("p (x d) -> p x d", d=32),
                    in0=vg[:, :Wd].rearrange("p (x d) -> p x d", d=32),
                    in1=_expand_last(ex[:, :T * 4], 32), op=OP.mult)
                den_ps = psum.tile([128, 4], F32, tag="den")
                for t in range(T):
                    nc.tensor.matmul(out=den_ps[:],
                                     lhsT=S[:, t * 128:(t + 1) * 128],
                                     rhs=ex[:, t * 4:(t + 1) * 4],
                                     start=(t == 0), stop=(t == T - 1))
                agg_ps = psum.tile([128, 128], F32, tag="agg")
                for t in range(T):
                    nc.tensor.matmul(out=agg_ps[:],
                                     lhsT=S[:, t * 128:(t + 1) * 128],
                                     rhs=vg[:, t * 128:(t + 1) * 128],
                                     start=(t == 0), stop=(t == T - 1))
                # ---- block epilogue ----
                rd = sb.tile([128, 4], F32, tag="rd")
                den_s = sb.tile([128, 4], F32, tag="den_s")
                nc.vector.tensor_scalar_add(den_s[:], den_ps[:], EPS)
                nc.vector.reciprocal(rd[:], den_s[:])
                aggn = sb.tile([128, 128], F32, tag="aggn")
                nc.vector.tensor_tensor(
                    out=aggn[:].rearrange("p (h d) -> p h d", d=32),
                    in0=agg_ps[:].rearrange("p (h d) -> p h d", d=32),
                    in1=_expand_last(rd[:], 32), op=OP.mult)
                g = sb.tile([128, 128], F32, tag="g")
                if gelu_mode == "hw":
                    nc.scalar.activation(g[:], aggn[:], AF.Gelu)
                else:
                    # sim-only tanh-approx gelu (CoreSim lacks Gelu/Erf)
                    t1 = sb.tile([128, 128], F32, tag="gel1")
                    nc.scalar.activation(t1[:], aggn[:], AF.Square)
                    nc.vector.tensor_tensor(out=t1[:], in0=t1[:], in1=aggn[:], op=OP.mult)
                    nc.vector.tensor_scalar_mul(t1[:], t1[:], 0.044715)
                    nc.vector.tensor_tensor(out=t1[:], in0=t1[:], in1=aggn[:], op=OP.add)
                    nc.scalar.activation(t1[:], t1[:], AF.Tanh, scale=0.7978845608028654)
                    nc.vector.tensor_scalar_add(t1[:], t1[:], 1.0)
                    nc.vector.tensor_tensor(out=t1[:], in0=t1[:], in1=aggn[:], op=OP.mult)
                    nc.vector.tensor_scalar_mul(g[:], t1[:], 0.5)
                gT_ps = psum.tile([128, 128], F32, tag="tr")
                nc.tensor.transpose(out=gT_ps[:], in_=g[:], identity=ident[:])
                gT = sb.tile([128, 128], F32, tag="gT")
                nc.vector.tensor_copy(gT[:], gT_ps[:])
                h_ps = psum.tile([128, 128], F32, tag="mm")
                nc.tensor.matmul(out=h_ps[:], lhsT=gT[:], rhs=W[f"Wo{l}"][:],
                                 start=True, stop=True)
                f2 = sb.tile([128, C], F32, tag="fblk2")
                nc.sync.dma_start(f2[:], src_feat[b * 128:(b + 1) * 128, :])
                hm = sb.tile([128, C], F32, tag="hm")
                nc.vector.tensor_scalar_mul(hm[:], h_ps[:], asig)
                nc.vector.tensor_tensor(out=hm[:], in0=hm[:], in1=W[f"boa{l}"][:],
                                        op=OP.add)
                xs_t = sb.tile([128, C], F32, tag="xs")
                nc.vector.tensor_scalar_mul(xs_t[:], f2[:], 1.0 - asig)
                nc.vector.tensor_tensor(out=hm[:], in0=hm[:], in1=xs_t[:],
                                        op=OP.add)
                nc.sync.dma_start(h_out[b * 128:(b + 1) * 128, :], hm[:])
                if l == 2:
                    pr = sb.tile([128, C], F32, tag="pr")
                    uv = sb.tile([128, 2], F32, tag="uv")
                    nc.vector.tensor_tensor(out=pr[:], in0=hm[:],
                                            in1=W["w1"][:], op=OP.mult)
                    nc.vector.tensor_reduce(out=uv[:, 0:1], in_=pr[:],
                                            axis=mybir.AxisListType.X, op=OP.add)
                    nc.vector.tensor_tensor(out=pr[:], in0=hm[:],
                                            in1=W["w2"][:], op=OP.mult)
                    nc.vector.tensor_reduce(out=uv[:, 1:2], in_=pr[:],
                                            axis=mybir.AxisListType.X, op=OP.add)
                    nc.sync.dma_start(uv_out[b * 128:(b + 1) * 128, :], uv[:])
                col += T

        layer(0, x_in, h1_dram[:], asig1)
        layer(1, h1_dram[:], z_shard[:], asig2)

    nc.compile()
    return nc


_CACHE = {}


def _get_program(meta, asig1, asig2, blp):
    key = (meta["N"], meta["E"], meta["P"], meta["T_b"], asig1, asig2, blp)
    if key not in _CACHE:
        _CACHE[key] = _build_program(meta, asig1, asig2, blp)
    return _CACHE[key]


def make_in_maps(inputs):
    inputs = {k: np.asarray(v) for k, v in inputs.items()}
    H, D = inputs["a1"].shape[0], inputs["a1"].shape[1]
    meta, arrays = _host_prep(inputs["x"].astype(np.float32),
                              inputs["edge_index"],
                              inputs["pos_edge_index"],
                              inputs["neg_edge_index"])
    w = _prep_weights(inputs, H, D)
    in_maps = []
    for c in range(CORES):
        m = dict(x_shard=arrays["xs"][c], ekv=arrays["ekv"][c],
                 eqr=arrays["eqr"][c], eslot=arrays["eslot"][c])
        for l in (1, 2):
            for n in (f"Wq{l}", f"Wk{l}", f"Wv{l}", f"Wo{l}",
                      f"bq{l}", f"bk{l}", f"bv{l}", f"boa{l}"):
                m[n] = w[n]
        m["w1"] = w["w1"]
        m["w2"] = w["w2"]
        in_maps.append(m)
    return meta, w, in_maps


def assemble(meta, results, inputs, blp):
    uv = np.concatenate([results[c]["uv_out"] for c in range(CORES)], axis=0)
    u1, u2 = uv[:, 0], uv[:, 1]
    pe, ne = inputs["pos_edge_index"], inputs["neg_edge_index"]
    pos = u1[pe[0]] + u2[pe[1]] + np.float32(blp)
    neg = u1[ne[0]] + u2[ne[1]] + np.float32(blp)
    return pos.astype(np.float32), neg.astype(np.float32)


def kernel(**inputs):
    meta, w, in_maps = make_in_maps(inputs)
    nc = _get_program(meta, w["asig1"], w["asig2"], w["blp"])
    res = bass_utils.run_bass_kernel_spmd(nc, in_maps,
                                          core_ids=list(range(CORES)))
    return assemble(meta, res.results, inputs, w["blp"])



# revision 17
# speedup vs baseline: 2.2153x; 2.2153x over previous
"""HGT link predictor on 8 Trainium2 NeuronCores (Bass/Tile SPMD kernel).

Strategy (hardcoded for nn_HGTLinkPredictor, N=50000 E=800000 P=100000 C=128 H=4 D=32):
 - Shard dst nodes (and their incoming edges) across 8 cores in contiguous
   128-node blocks; edges sorted by dst on host.
 - Features flow in fp16. Node features are kept TRANSPOSED ([C, n]) in SBUF
   so q/k/v projections are a single 384-wide matmul per 128-node block with
   no on-device transposes; relation transforms + attention scale are folded
   into the weights on host.
 - k and v rows are concatenated ([N, 256] fp16); per-edge rows are fetched
   with gpsimd.dma_gather (one instruction per dst block per kv half, int16
   indices, ~1us fixed SWDGE cost amortized over the whole block) instead of
   per-128-row indirect DMAs. The kv table is split in two halves so row
   indices fit int16; each block's edges are reordered low-half-first on the
   host.
 - Segment softmax/weighted-sum per 128-node block via one-hot selection
   matrices on the PE; the denominator is computed in the same matmul chain
   (ex appended as 4 extra rhs columns) and division deferred to the block
   epilogue. alpha is clamped at 11 so exp() fits fp16.
 - The edge phase is split into two passes per layer so the scalar engine
   activation table is not thrashed between Exp and Gelu per block.
 - Epilogue is done transposed (lhsT=Wo trick) so h1^T stays in SBUF for
   layer 2 and the link decode is a [C,2]-stationary matmul per block.
"""

import math
import os
import numpy as np
from contextlib import ExitStack

import concourse.bass as bass
import concourse.tile as tile
from concourse import bacc, mybir
from concourse import bass_utils
from concourse.masks import make_identity
from concourse import library_config

F32 = mybir.dt.float32
F16 = mybir.dt.float16
I16 = mybir.dt.int16
AF = mybir.ActivationFunctionType
OP = mybir.AluOpType

CORES = 8
EPS = 1e-30
ACLAMP = 11.0


def _v(ap, off, dims):
    """Custom free-dim view of a 2D [part, width] AP: keep partition dim,
    replace free dims with `dims` ([step, num] pairs), add `off` elements."""
    return bass.AP(ap.tensor, ap.offset + off, [list(ap.ap[0])] + [list(d) for d in dims])


def _wrap16(flat):
    """[M*16] -> [16, M] with element i at [i%16, i//16]."""
    return flat.reshape(-1, 16).T.copy()


# ----------------------------------------------------------------- host prep

def _host_prep(x, edge_index, pos_edge_index, neg_edge_index):
    N, C = x.shape
    E = edge_index.shape[1]
    P = pos_edge_index.shape[1]

    NPC = int(math.ceil(N / (CORES * 128))) * 128   # nodes per core (padded)
    BPC = NPC // 128                                # blocks per core
    NPAD = NPC * CORES
    HALF = NPAD // 2
    assert HALF < 2 ** 15 and NPC < 2 ** 15

    src = edge_index[0].astype(np.int64)
    dst = edge_index[1].astype(np.int64)
    order = np.argsort(dst, kind="stable")
    s_src, s_dst = src[order], dst[order]

    core_of = s_dst // NPC
    blk_of = (s_dst % NPC) // 128
    gblk = core_of * BPC + blk_of
    ishigh = (s_src >= HALF).astype(np.int64)

    # reorder within each (core, block): low-half src first
    order2 = np.argsort(gblk * 2 + ishigh, kind="stable")
    s_src, s_dst = s_src[order2], s_dst[order2]
    core_of, blk_of, gblk, ishigh = (core_of[order2], blk_of[order2],
                                     gblk[order2], ishigh[order2])

    # per (core, block, half) counts -> shared tile counts per block index
    cnt = np.zeros((CORES, BPC, 2), dtype=np.int64)
    np.add.at(cnt, (core_of, blk_of, ishigh), 1)
    T1_b = np.ceil(cnt[:, :, 0].max(axis=0) / 128).astype(np.int64)  # [BPC]
    T2_b = np.ceil(cnt[:, :, 1].max(axis=0) / 128).astype(np.int64)
    empty = (T1_b + T2_b) == 0
    T1_b[empty] = 1
    T_b = T1_b + T2_b
    tiles_total = int(T_b.sum())
    tile_start = np.concatenate([[0], np.cumsum(T_b)])[:-1]          # [BPC]

    # rank of each edge within its (core, block, half) group
    ghalf = gblk * 2 + ishigh
    grp_start = np.zeros(CORES * BPC * 2 + 1, dtype=np.int64)
    np.add.at(grp_start, ghalf + 1, 1)
    grp_start = np.cumsum(grp_start)
    pos_in_grp = np.arange(E) - grp_start[ghalf]

    # flat slot within the core's [tiles_total*128] edge array
    flat_pos = (tile_start[blk_of] * 128 + ishigh * T1_b[blk_of] * 128
                + pos_in_grp)

    cap = tiles_total * 128
    kvidx = np.zeros((CORES, cap), dtype=np.int16)
    qidx = np.zeros((CORES, cap), dtype=np.int16)
    eslot = np.full((CORES, cap), -1.0, dtype=np.float16)

    np_c = core_of
    kvidx[np_c, flat_pos] = (s_src - ishigh * HALF).astype(np.int16)
    qidx[np_c, flat_pos] = (s_dst - np_c * NPC).astype(np.int16)
    eslot[np_c, flat_pos] = (s_dst % 128).astype(np.float16)

    # eslot -> [128, tiles_total] partition-major; idxs -> [16->128, tiles*8]
    eslot = eslot.reshape(CORES, tiles_total, 128).transpose(0, 2, 1).copy()
    kv16 = np.zeros((CORES, 128, tiles_total * 8), dtype=np.int16)
    q16 = np.zeros((CORES, 128, tiles_total * 8), dtype=np.int16)
    for c in range(CORES):
        # the SWDGE ucode reads the [16, M] wrap from partition group
        # 2*queue_num(+1); replicate everywhere so any queue works
        kv16[c] = np.tile(_wrap16(kvidx[c]), (8, 1))
        q16[c] = np.tile(_wrap16(qidx[c]), (8, 1))

    # x shards, transposed: [C, NPC] fp16
    xpad = np.zeros((NPAD, C), dtype=np.float32)
    xpad[:N] = x
    xT = np.zeros((CORES, C, NPC), dtype=np.float16)
    for c in range(CORES):
        xT[c] = xpad[c * NPC:(c + 1) * NPC].T.astype(np.float16)

    meta = dict(N=N, C=C, E=E, P=P, NPC=NPC, BPC=BPC, NPAD=NPAD, HALF=HALF,
                T1_b=tuple(int(t) for t in T1_b),
                T2_b=tuple(int(t) for t in T2_b),
                tiles_total=tiles_total)
    arrays = dict(kv16=kv16, q16=q16, eslot=eslot, xT=xT)
    return meta, arrays


def _prep_weights(inputs, H, D):
    """Fold relation transforms + attention scale into the linear weights."""
    C = inputs["W1k"].shape[0]
    out = {}
    for l in (1, 2):
        a_rel = np.asarray(inputs[f"a{l}"], np.float64)
        m_rel = np.asarray(inputs[f"m{l}"], np.float64)
        p_rel = np.asarray(inputs[f"p{l}"], np.float64)
        A = np.zeros((C, C)); M = np.zeros((C, C))
        for h in range(H):
            A[h * D:(h + 1) * D, h * D:(h + 1) * D] = a_rel[h]
            M[h * D:(h + 1) * D, h * D:(h + 1) * D] = m_rel[h]
        qscale = np.repeat(p_rel / np.sqrt(D), D)
        Wq = np.asarray(inputs[f"W{l}q"], np.float64) * qscale
        bq = np.asarray(inputs[f"b{l}q"], np.float64) * qscale
        Wk = np.asarray(inputs[f"W{l}k"], np.float64) @ A
        bk = np.asarray(inputs[f"b{l}k"], np.float64) @ A
        Wv = np.asarray(inputs[f"W{l}v"], np.float64) @ M
        bv = np.asarray(inputs[f"b{l}v"], np.float64) @ M
        a_sig = float(1.0 / (1.0 + np.exp(-float(inputs[f"skip{l}"]))))
        Wqkv = np.concatenate([Wq, Wk, Wv], axis=1)        # [C, 384]
        bqkv = np.concatenate([bq, bk, bv])                # [384]
        out[f"Wqkv{l}"] = Wqkv.astype(np.float16)
        out[f"bqkv{l}"] = np.broadcast_to(bqkv.astype(np.float32), (128, 3 * C)).copy()
        out[f"Wo{l}"] = np.asarray(inputs[f"Wo{l}"], np.float16)
        out[f"boaT{l}"] = (a_sig * np.asarray(inputs[f"bo{l}"], np.float64)
                           ).astype(np.float32).reshape(C, 1).copy()
        out[f"asig{l}"] = a_sig
    Wlp = np.asarray(inputs["Wlp"], np.float32)
    out["w12"] = np.stack([Wlp[:C, 0], Wlp[C:, 0]], axis=1).astype(np.float16)  # [C,2]
    out["blp"] = float(np.asarray(inputs["blp"]).reshape(-1)[0])
    return out


# ------------------------------------------------------------------- program

def _build_program(meta, asig1, asig2, gelu_mode="hw", shared_kvf=True,
                   nqueues=1):
    NPC, BPC, NPAD, HALF = meta["NPC"], meta["BPC"], meta["NPAD"], meta["HALF"]
    T1_b, T2_b = meta["T1_b"], meta["T2_b"]
    tiles_total = meta["tiles_total"]
    T_b = [a + b for a, b in zip(T1_b, T2_b)]
    Tmax = max(T_b)
    C = meta["C"]

    nc = bacc.Bacc("TRN2", target_bir_lowering=False, debug=False,
                   num_devices=CORES, num_swdge_queues=nqueues)

    # --- I/O -------------------------------------------------------------
    xT_in = nc.dram_tensor("xT", [C, NPC], F16, kind="ExternalInput").ap()
    kv16_in = nc.dram_tensor("kv16", [128, tiles_total * 8], I16,
                             kind="ExternalInput").ap()
    q16_in = nc.dram_tensor("q16", [128, tiles_total * 8], I16,
                            kind="ExternalInput").ap()
    eslot_in = nc.dram_tensor("eslot", [128, tiles_total], F16,
                              kind="ExternalInput").ap()
    w_specs = [("Wqkv1", [C, 3 * C], F16), ("Wqkv2", [C, 3 * C], F16),
               ("bqkv1", [128, 3 * C], F32), ("bqkv2", [128, 3 * C], F32),
               ("Wo1", [C, C], F16), ("Wo2", [C, C], F16),
               ("boaT1", [C, 1], F32), ("boaT2", [C, 1], F32),
               ("w12", [C, 2], F16)]
    w_in = {n: nc.dram_tensor(n, shp, dt, kind="ExternalInput").ap()
            for (n, shp, dt) in w_specs}
    uv_out = nc.dram_tensor("uvT_out", [2, NPC], F32, kind="ExternalOutput").ap()
    debug = os.environ.get("HGT_DEBUG", "0") == "1"
    dbg = {}
    if debug:
        for n, shp, dt in [("d_kvg", [128, Tmax * 256], F16),
                           ("d_qg", [128, Tmax * 128], F16),
                           ("d_S", [128, Tmax * 128], F16),
                           ("d_kq", [128, Tmax * 128], F16),
                           ("d_alpha", [128, Tmax * 4], F32),
                           ("d_ex", [128, Tmax * 4], F16),
                           ("d_r", [128, Tmax * 132], F16),
                           ("d_aggsb", [128, 132 * 4], F32),
                           ("d_h1T", [128, NPC], F16)]:
            dbg[n] = nc.dram_tensor(n, shp, dt, kind="ExternalOutput").ap()

    with tile.TileContext(nc) as tc, ExitStack() as ctx:
        sb = ctx.enter_context(tc.tile_pool(name="sb", bufs=3))
        sbs = ctx.enter_context(tc.tile_pool(name="sbs", bufs=3))
        cpool = ctx.enter_context(tc.tile_pool(name="const", bufs=1))
        psA = ctx.enter_context(tc.tile_pool(name="psA", bufs=2, space="PSUM"))
        psB = ctx.enter_context(tc.tile_pool(name="psB", bufs=1, space="PSUM"))
        dram = ctx.enter_context(tc.tile_pool(name="dr", bufs=1, space="DRAM"))

        # --- constants into SBUF ----------------------------------------
        W = {}
        for (n, shp, dt) in w_specs:
            W[n] = cpool.tile(shp, dt, tag=f"w_{n}", name=f"wt_{n}")
            nc.sync.dma_start(W[n][:], w_in[n][:])
        kv16_sb = cpool.tile([128, tiles_total * 8], I16, tag="kv16")
        nc.sync.dma_start(kv16_sb[:], kv16_in[:])
        q16_sb = cpool.tile([128, tiles_total * 8], I16, tag="q16")
        nc.sync.dma_start(q16_sb[:], q16_in[:])
        eslot_sb = cpool.tile([128, tiles_total], F16, tag="eslot")
        nc.sync.dma_start(eslot_sb[:], eslot_in[:])
        xT_sb = cpool.tile([C, NPC], F16, tag="xT")
        nc.sync.dma_start(xT_sb[:], xT_in[:])

        ident = cpool.tile([128, 128], F16, tag="ident")
        make_identity(nc, ident[:])
        iota_f = cpool.tile([128, Tmax * 128], F16, tag="iota_f")
        iota_i, free_iota = tc.tile([128, Tmax * 128], mybir.dt.int32,
                                    name="iota_i")
        nc.gpsimd.iota(iota_i[:], pattern=[[0, Tmax], [1, 128]], base=0,
                       channel_multiplier=0)
        nc.vector.tensor_copy(iota_f[:], iota_i[:])
        free_iota()
        # dma_gather lives in the 'mlp' GPSIMD ucode library; the iota /
        # identity setup above needs the default library, so swap after.
        nc.gpsimd.load_library(library_config.mlp)

        h1T = cpool.tile([C, NPC], F16, tag="h1T")
        aggn_all = cpool.tile([128, BPC * 128], F16, tag="aggn_all")

        # --- DRAM scratch ------------------------------------------------
        q_dram = dram.tile([NPC, C], F16, tag="qd", name="q_dram")
        kv_shard = dram.tile([NPC, 2 * C], F16, tag="kvs", name="kv_shard")
        kvf_kw = dict(addr_space="Shared") if shared_kvf else {}
        kv_full = [dram.tile([NPAD, 2 * C], F16, tag=f"kvf{l}", name=f"kv_full{l}",
                             **kvf_kw) for l in (0, 1)]

        def layer(li, srcT, asig):
            l = li + 1
            kvf = kv_full[li]
            # ---- projections: one matmul per block ----
            for b in range(BPC):
                blk = slice(b * 128, (b + 1) * 128)
                ps = psA.tile([128, 3 * C], F32, tag="proj")
                nc.tensor.matmul(out=ps[:], lhsT=srcT[:, blk], rhs=W[f"Wqkv{l}"][:],
                                 start=True, stop=True)
                qkv = sb.tile([128, 3 * C], F16, tag="qkv")
                nc.vector.tensor_tensor(out=qkv[:], in0=ps[:], in1=W[f"bqkv{l}"][:],
                                        op=OP.add)
                nc.sync.dma_start(q_dram[blk, :], qkv[:, 0:C])
                nc.sync.dma_start(kv_shard[blk, :], qkv[:, C:3 * C])
            # ---- exchange k/v ----
            nc.gpsimd.collective_compute(
                "AllGather", OP.bypass,
                replica_groups=[list(range(CORES))],
                ins=[kv_shard[:]], outs=[kvf[:]])
            # ---- edge pass A: gather + attention + aggregate ----
            # dma_gather breaks above ~1024 idxs/instruction; chunk to 8 tiles
            def gather_rows(dst, dst_off, table, idx_sb, col8, ntiles, elem, qn):
                done = 0
                while done < ntiles:
                    k = min(8, ntiles - done)
                    nc.gpsimd.dma_gather(
                        out_ap=_v(dst[:], dst_off + done * elem,
                                  [[elem, k], [1, elem]]),
                        in_ap=table,
                        idxs_ap=idx_sb[:, (col8 + done) * 8:(col8 + done + k) * 8],
                        num_idxs=k * 128, num_idxs_reg=k * 128,
                        elem_size=elem, queue_num=qn)
                    done += k

            col = 0
            for b in range(BPC):
                T1, T2 = T1_b[b], T2_b[b]
                T = T1 + T2
                qn = b % nqueues
                kvg = sb.tile([128, Tmax * 256], F16, tag="kvg")
                if T1:
                    gather_rows(kvg, 0, kvf[0:HALF, :], kv16_sb, col, T1, 256, qn)
                if T2:
                    gather_rows(kvg, T1 * 256, kvf[HALF:NPAD, :], kv16_sb,
                                col + T1, T2, 256, qn)
                qg = sb.tile([128, Tmax * 128], F16, tag="qg")
                gather_rows(qg, 0, q_dram[:], q16_sb, col, T, 128, qn)
                S = sb.tile([128, Tmax * 128], F16, tag="S")
                nc.vector.tensor_tensor(
                    out=_v(S[:], 0, [[128, T], [1, 128]]),
                    in0=_v(iota_f[:], 0, [[128, T], [1, 128]]),
                    in1=_v(eslot_sb[:], col, [[1, T], [0, 128]]),
                    op=OP.is_equal)
                kq = sb.tile([128, Tmax * 128], F16, tag="kq")
                nc.vector.tensor_tensor(
                    out=_v(kq[:], 0, [[128, T], [1, 128]]),
                    in0=_v(kvg[:], 0, [[256, T], [1, 128]]),
                    in1=_v(qg[:], 0, [[128, T], [1, 128]]),
                    op=OP.mult)
                alpha = sbs.tile([128, Tmax * 4], F32, tag="alpha")
                nc.vector.tensor_reduce(
                    out=alpha[:, :T * 4],
                    in_=_v(kq[:], 0, [[32, T * 4], [1, 32]]),
                    axis=mybir.AxisListType.X, op=OP.add)
                alc = sbs.tile([128, Tmax * 4], F32, tag="alc")
                nc.vector.tensor_scalar_min(alc[:, :T * 4], alpha[:, :T * 4], ACLAMP)
                ex = sbs.tile([128, Tmax * 4], F16, tag="ex")
                nc.scalar.activation(ex[:, :T * 4], alc[:, :T * 4], AF.Exp)
                r = sb.tile([128, Tmax * 132], F16, tag="r")
                nc.vector.tensor_tensor(
                    out=_v(r[:], 0, [[132, T], [32, 4], [1, 32]]),
                    in0=_v(kvg[:], 128, [[256, T], [32, 4], [1, 32]]),
                    in1=_v(ex[:], 0, [[4, T], [1, 4], [0, 32]]),
                    op=OP.mult)
                nc.vector.tensor_copy(
                    out=_v(r[:], 128, [[132, T], [1, 4]]),
                    in_=_v(ex[:], 0, [[4, T], [1, 4]]))
                agg = psA.tile([128, 132], F32, tag="agg")
                for t in range(T):
                    nc.tensor.matmul(out=agg[:],
                                     lhsT=S[:, t * 128:(t + 1) * 128],
                                     rhs=r[:, t * 132:(t + 1) * 132],
                                     start=(t == 0), stop=(t == T - 1))
                rds = sbs.tile([128, 4], F32, tag="rds")
                nc.vector.tensor_scalar_add(rds[:], agg[:, 128:132], EPS)
                rd = sbs.tile([128, 4], F32, tag="rd")
                nc.vector.reciprocal(rd[:], rds[:])
                nc.vector.tensor_tensor(
                    out=_v(aggn_all[:], b * 128, [[32, 4], [1, 32]]),
                    in0=_v(agg[:], 0, [[32, 4], [1, 32]]),
                    in1=_v(rd[:], 0, [[1, 4], [0, 32]]),
                    op=OP.mult)
                if debug and li == 0 and b == 0:
                    for name, t in (("d_kvg", kvg), ("d_qg", qg), ("d_S", S),
                                    ("d_kq", kq), ("d_alpha", alpha),
                                    ("d_ex", ex), ("d_r", r)):
                        nc.sync.dma_start(dbg[name][:], t[:])
                col += T
            # ---- edge pass B: gelu + output proj + skip ----
            for b in range(BPC):
                blk = slice(b * 128, (b + 1) * 128)
                anT = psB.tile([128, 128], F16, tag="anT")
                nc.tensor.transpose(out=anT[:], in_=aggn_all[:, blk],
                                    identity=ident[:])
                gT = sbs.tile([128, 128], F16, tag="gT")
                if gelu_mode == "hw":
                    nc.scalar.activation(gT[:], anT[:], AF.Gelu)
                else:
                    # sim-only tanh-approx gelu (CoreSim lacks Gelu/Erf)
                    t1 = sbs.tile([128, 128], F32, tag="gel1")
                    nc.scalar.activation(t1[:], anT[:], AF.Square)
                    nc.vector.tensor_tensor(out=t1[:], in0=t1[:], in1=anT[:], op=OP.mult)
                    nc.vector.tensor_scalar_mul(t1[:], t1[:], 0.044715)
                    nc.vector.tensor_tensor(out=t1[:], in0=t1[:], in1=anT[:], op=OP.add)
                    nc.scalar.activation(t1[:], t1[:], AF.Tanh, scale=0.7978845608028654)
                    nc.vector.tensor_scalar_add(t1[:], t1[:], 1.0)
                    nc.vector.tensor_tensor(out=t1[:], in0=t1[:], in1=anT[:], op=OP.mult)
                    nc.vector.tensor_scalar_mul(gT[:], t1[:], 0.5)
                hps = psB.tile([128, 128], F32, tag="hps")
                nc.tensor.matmul(out=hps[:], lhsT=W[f"Wo{l}"][:], rhs=gT[:],
                                 start=True, stop=True)
                ha = sbs.tile([128, 128], F16, tag="ha")
                nc.scalar.activation(ha[:], hps[:], AF.Identity,
                                     bias=W[f"boaT{l}"][:], scale=asig)
                if l == 1:
                    nc.vector.scalar_tensor_tensor(
                        out=h1T[:, blk], in0=srcT[:, blk], scalar=1.0 - asig,
                        in1=ha[:], op0=OP.mult, op1=OP.add)
                else:
                    hm = sbs.tile([128, 128], F16, tag="hm")
                    nc.vector.scalar_tensor_tensor(
                        out=hm[:], in0=srcT[:, blk], scalar=1.0 - asig,
                        in1=ha[:], op0=OP.mult, op1=OP.add)
                    uvp = psB.tile([2, 128], F32, tag="uvp")
                    nc.tensor.matmul(out=uvp[:], lhsT=W["w12"][:], rhs=hm[:],
                                     start=True, stop=True)
                    uvt = sbs.tile([2, 128], F32, tag="uvt")
                    nc.vector.tensor_copy(uvt[:], uvp[:])
                    nc.sync.dma_start(uv_out[:, blk], uvt[:])

        layer(0, xT_sb[:], asig1)
        if debug:
            nc.sync.dma_start(dbg["d_aggsb"][:], aggn_all[:, :132 * 4])
            nc.sync.dma_start(dbg["d_h1T"][:], h1T[:])
        layer(1, h1T[:], asig2)

    nc.compile()
    return nc


_CACHE = {}


def _get_program(meta, asig1, asig2, blp, gelu_mode=None, shared_kvf=None,
                 nqueues=None):
    if gelu_mode is None:
        gelu_mode = os.environ.get("HGT_GELU", "hw")
    if shared_kvf is None:
        shared_kvf = os.environ.get("HGT_SHARED_KVF", "1") == "1"
    if nqueues is None:
        nqueues = int(os.environ.get("HGT_NQUEUES", "1"))
    key = (meta["N"], meta["E"], meta["P"], meta["T1_b"], meta["T2_b"],
           asig1, asig2, gelu_mode, shared_kvf, nqueues)
    if key not in _CACHE:
        _CACHE[key] = _build_program(meta, asig1, asig2, gelu_mode, shared_kvf,
                                     nqueues)
    return _CACHE[key]


def make_in_maps(inputs):
    inputs = {k: np.asarray(v) for k, v in inputs.items()}
    H, D = inputs["a1"].shape[0], inputs["a1"].shape[1]
    meta, arrays = _host_prep(inputs["x"].astype(np.float32),
                              inputs["edge_index"],
                              inputs["pos_edge_index"],
                              inputs["neg_edge_index"])
    w = _prep_weights(inputs, H, D)
    in_maps = []
    for c in range(CORES):
        m = dict(xT=arrays["xT"][c], kv16=arrays["kv16"][c],
                 q16=arrays["q16"][c], eslot=arrays["eslot"][c])
        for n in ("Wqkv1", "Wqkv2", "bqkv1", "bqkv2", "Wo1", "Wo2",
                  "boaT1", "boaT2", "w12"):
            m[n] = w[n]
        in_maps.append(m)
    return meta, w, in_maps


def assemble(meta, results, inputs, blp):
    uv = np.concatenate([results[c]["uvT_out"] for c in range(CORES)], axis=1)
    u1, u2 = uv[0], uv[1]
    pe, ne = inputs["pos_edge_index"], inputs["neg_edge_index"]
    pos = u1[pe[0]] + u2[pe[1]] + np.float32(blp)
    neg = u1[ne[0]] + u2[ne[1]] + np.float32(blp)
    return pos.astype(np.float32), neg.astype(np.float32)


def kernel(**inputs):
    meta, w, in_maps = make_in_maps(inputs)
    nc = _get_program(meta, w["asig1"], w["asig2"], w["blp"])
    res = bass_utils.run_bass_kernel_spmd(nc, in_maps,
                                          core_ids=list(range(CORES)))
    return assemble(meta, res.results, inputs, w["blp"])


# revision 21
# speedup vs baseline: 4.1428x; 1.8701x over previous
"""HGT link predictor on 8 Trainium2 NeuronCores (Bass/Tile SPMD kernel).

Strategy (hardcoded for nn_HGTLinkPredictor, N=50000 E=800000 P=100000 C=128 H=4 D=32):
 - Shard dst nodes (and their incoming edges) across 8 cores in contiguous
   128-node blocks; edges sorted by dst on host.
 - Features flow in fp16. Node features are kept TRANSPOSED ([C, n]) in SBUF
   so q/k/v projections are a single 384-wide matmul per 128-node block with
   no on-device transposes; relation transforms + attention scale are folded
   into the weights on host.
 - k and v rows are concatenated ([N, 256] fp16); per-edge rows are fetched
   with gpsimd.dma_gather (<=1024 int16 indices per instruction, ~5us each,
   amortized over 8 tiles) instead of per-128-row indirect DMAs. The kv table
   is split in two halves so row indices fit int16; each block's edges are
   reordered low-half-first on the host.
 - q is never round-tripped through DRAM or gathered: q rows stay in SBUF
   ([n, c] per block) and per-edge q is expanded on the tensor engine with
   host-precomputed one-hot selection matrices (qg = ST_t.T @ q_blk).
 - Segment softmax/weighted-sum per 128-node block via the same one-hot
   matrices on the PE; the denominator rides along as 4 extra rhs columns
   and division is deferred to the block epilogue. alpha is clamped at 11
   so exp() fits fp16.
 - The edge phase is split into two passes per layer so the scalar engine
   activation table is not thrashed between Exp and Gelu per block.
 - Epilogue is done transposed (lhsT=Wo trick) so h1^T stays in SBUF for
   layer 2 and the link decode is a [C,2]-stationary matmul per block.
"""

import math
import os
import numpy as np
from contextlib import ExitStack

import concourse.bass as bass
import concourse.tile as tile
from concourse import bacc, mybir
from concourse import bass_utils
from concourse.masks import make_identity
from concourse import library_config

F32 = mybir.dt.float32
F16 = mybir.dt.float16
I16 = mybir.dt.int16
AF = mybir.ActivationFunctionType
OP = mybir.AluOpType

CORES = 8
EPS = 1e-30
ACLAMP = 11.0
GCHUNK = 8          # dma_gather tiles per instruction (1024 idxs)


def _v(ap, off, dims):
    """Custom free-dim view of a 2D [part, width] AP: keep partition dim,
    replace free dims with `dims` ([step, num] pairs), add `off` elements."""
    return bass.AP(ap.tensor, ap.offset + off, [list(ap.ap[0])] + [list(d) for d in dims])


def _wrap16(flat):
    """[M*16] -> [16, M] with element i at [i%16, i//16]."""
    return flat.reshape(-1, 16).T.copy()


# ----------------------------------------------------------------- host prep

def _host_prep(x, edge_index, pos_edge_index, neg_edge_index):
    N, C = x.shape
    E = edge_index.shape[1]
    P = pos_edge_index.shape[1]

    NPC = int(math.ceil(N / (CORES * 128))) * 128   # nodes per core (padded)
    BPC = NPC // 128                                # blocks per core
    NPAD = NPC * CORES
    HALF = NPAD // 2
    assert HALF < 2 ** 15

    src = edge_index[0].astype(np.int64)
    dst = edge_index[1].astype(np.int64)
    order = np.argsort(dst, kind="stable")
    s_src, s_dst = src[order], dst[order]

    core_of = s_dst // NPC
    blk_of = (s_dst % NPC) // 128
    gblk = core_of * BPC + blk_of
    ishigh = (s_src >= HALF).astype(np.int64)

    # reorder within each (core, block): low-half src first
    order2 = np.argsort(gblk * 2 + ishigh, kind="stable")
    s_src, s_dst = s_src[order2], s_dst[order2]
    core_of, blk_of, gblk, ishigh = (core_of[order2], blk_of[order2],
                                     gblk[order2], ishigh[order2])

    # per (core, block, half) counts -> shared tile counts per block index
    cnt = np.zeros((CORES, BPC, 2), dtype=np.int64)
    np.add.at(cnt, (core_of, blk_of, ishigh), 1)
    T1_b = np.ceil(cnt[:, :, 0].max(axis=0) / 128).astype(np.int64)  # [BPC]
    T2_b = np.ceil(cnt[:, :, 1].max(axis=0) / 128).astype(np.int64)
    empty = (T1_b + T2_b) == 0
    T1_b[empty] = 1
    T_b = T1_b + T2_b
    tiles_total = int(T_b.sum())
    tile_start = np.concatenate([[0], np.cumsum(T_b)])[:-1]          # [BPC]

    # rank of each edge within its (core, block, half) group
    ghalf = gblk * 2 + ishigh
    grp_start = np.zeros(CORES * BPC * 2 + 1, dtype=np.int64)
    np.add.at(grp_start, ghalf + 1, 1)
    grp_start = np.cumsum(grp_start)
    pos_in_grp = np.arange(E) - grp_start[ghalf]

    # flat slot within the core's [tiles_total*128] edge array
    flat_pos = (tile_start[blk_of] * 128 + ishigh * T1_b[blk_of] * 128
                + pos_in_grp)

    cap = tiles_total * 128
    kvidx = np.zeros((CORES, cap), dtype=np.int16)
    eslot = np.full((CORES, cap), -1, dtype=np.int64)

    kvidx[core_of, flat_pos] = (s_src - ishigh * HALF).astype(np.int16)
    eslot[core_of, flat_pos] = s_dst % 128

    # one-hot selection matrices, [128, tiles_total*128] fp16
    #   S[p, t*128 + n]  = (eslot[edge t*128+p] == n)
    #   ST[n, t*128 + p] = (eslot[edge t*128+p] == n)
    S = np.zeros((CORES, 128, tiles_total * 128), dtype=np.float16)
    ST = np.zeros((CORES, 128, tiles_total * 128), dtype=np.float16)
    for c in range(CORES):
        i = np.arange(cap)
        valid = eslot[c] >= 0
        iv, sl = i[valid], eslot[c][valid]
        S[c, iv % 128, (iv // 128) * 128 + sl] = 1.0
        ST[c, sl, iv] = 1.0

    kv16 = np.zeros((CORES, 128, tiles_total * 8), dtype=np.int16)
    for c in range(CORES):
        # the SWDGE ucode reads the [16, M] wrap from partition group
        # 2*queue_num(+1); replicate everywhere so any queue works
        kv16[c] = np.tile(_wrap16(kvidx[c]), (8, 1))

    # x shards, transposed: [C, NPC] fp16
    xpad = np.zeros((NPAD, C), dtype=np.float32)
    xpad[:N] = x
    xT = np.zeros((CORES, C, NPC), dtype=np.float16)
    for c in range(CORES):
        xT[c] = xpad[c * NPC:(c + 1) * NPC].T.astype(np.float16)

    meta = dict(N=N, C=C, E=E, P=P, NPC=NPC, BPC=BPC, NPAD=NPAD, HALF=HALF,
                T1_b=tuple(int(t) for t in T1_b),
                T2_b=tuple(int(t) for t in T2_b),
                tiles_total=tiles_total)
    arrays = dict(kv16=kv16, S=S, ST=ST, xT=xT)
    return meta, arrays


def _prep_weights(inputs, H, D):
    """Fold relation transforms + attention scale into the linear weights."""
    C = inputs["W1k"].shape[0]
    out = {}
    for l in (1, 2):
        a_rel = np.asarray(inputs[f"a{l}"], np.float64)
        m_rel = np.asarray(inputs[f"m{l}"], np.float64)
        p_rel = np.asarray(inputs[f"p{l}"], np.float64)
        A = np.zeros((C, C)); M = np.zeros((C, C))
        for h in range(H):
            A[h * D:(h + 1) * D, h * D:(h + 1) * D] = a_rel[h]
            M[h * D:(h + 1) * D, h * D:(h + 1) * D] = m_rel[h]
        qscale = np.repeat(p_rel / np.sqrt(D), D)
        Wq = np.asarray(inputs[f"W{l}q"], np.float64) * qscale
        bq = np.asarray(inputs[f"b{l}q"], np.float64) * qscale
        Wk = np.asarray(inputs[f"W{l}k"], np.float64) @ A
        bk = np.asarray(inputs[f"b{l}k"], np.float64) @ A
        Wv = np.asarray(inputs[f"W{l}v"], np.float64) @ M
        bv = np.asarray(inputs[f"b{l}v"], np.float64) @ M
        a_sig = float(1.0 / (1.0 + np.exp(-float(inputs[f"skip{l}"]))))
        Wqkv = np.concatenate([Wq, Wk, Wv], axis=1)        # [C, 384]
        bqkv = np.concatenate([bq, bk, bv])                # [384]
        out[f"Wqkv{l}"] = Wqkv.astype(np.float16)
        out[f"bqkv{l}"] = np.broadcast_to(bqkv.astype(np.float32), (128, 3 * C)).copy()
        out[f"Wo{l}"] = np.asarray(inputs[f"Wo{l}"], np.float16)
        out[f"boaT{l}"] = (a_sig * np.asarray(inputs[f"bo{l}"], np.float64)
                           ).astype(np.float32).reshape(C, 1).copy()
        out[f"asig{l}"] = a_sig
    Wlp = np.asarray(inputs["Wlp"], np.float32)
    out["w12"] = np.stack([Wlp[:C, 0], Wlp[C:, 0]], axis=1).astype(np.float16)  # [C,2]
    out["blp"] = float(np.asarray(inputs["blp"]).reshape(-1)[0])
    return out


# ------------------------------------------------------------------- program

def _build_program(meta, asig1, asig2, gelu_mode="hw", shared_kvf=True,
                   nqueues=1):
    NPC, BPC, NPAD, HALF = meta["NPC"], meta["BPC"], meta["NPAD"], meta["HALF"]
    T1_b, T2_b = meta["T1_b"], meta["T2_b"]
    tiles_total = meta["tiles_total"]
    T_b = [a + b for a, b in zip(T1_b, T2_b)]
    Tmax = max(T_b)
    C = meta["C"]

    nc = bacc.Bacc("TRN2", target_bir_lowering=False, debug=False,
                   num_devices=CORES, num_swdge_queues=nqueues)

    # --- I/O -------------------------------------------------------------
    xT_in = nc.dram_tensor("xT", [C, NPC], F16, kind="ExternalInput").ap()
    kv16_in = nc.dram_tensor("kv16", [128, tiles_total * 8], I16,
                             kind="ExternalInput").ap()
    S_in = nc.dram_tensor("S_hot", [128, tiles_total * 128], F16,
                          kind="ExternalInput").ap()
    ST_in = nc.dram_tensor("ST_hot", [128, tiles_total * 128], F16,
                           kind="ExternalInput").ap()
    w_specs = [("Wqkv1", [C, 3 * C], F16), ("Wqkv2", [C, 3 * C], F16),
               ("bqkv1", [128, 3 * C], F32), ("bqkv2", [128, 3 * C], F32),
               ("Wo1", [C, C], F16), ("Wo2", [C, C], F16),
               ("boaT1", [C, 1], F32), ("boaT2", [C, 1], F32),
               ("w12", [C, 2], F16)]
    w_in = {n: nc.dram_tensor(n, shp, dt, kind="ExternalInput").ap()
            for (n, shp, dt) in w_specs}
    uv_out = nc.dram_tensor("uvT_out", [2, NPC], F32, kind="ExternalOutput").ap()

    with tile.TileContext(nc) as tc, ExitStack() as ctx:
        sb = ctx.enter_context(tc.tile_pool(name="sb", bufs=3))
        sbs = ctx.enter_context(tc.tile_pool(name="sbs", bufs=3))
        cpool = ctx.enter_context(tc.tile_pool(name="const", bufs=1))
        psA = ctx.enter_context(tc.tile_pool(name="psA", bufs=1, space="PSUM"))
        psQ = ctx.enter_context(tc.tile_pool(name="psQ", bufs=1, space="PSUM"))
        psB = ctx.enter_context(tc.tile_pool(name="psB", bufs=1, space="PSUM"))
        dram = ctx.enter_context(tc.tile_pool(name="dr", bufs=1, space="DRAM"))

        # --- constants into SBUF ----------------------------------------
        W = {}
        for (n, shp, dt) in w_specs:
            W[n] = cpool.tile(shp, dt, tag=f"w_{n}", name=f"wt_{n}")
            nc.sync.dma_start(W[n][:], w_in[n][:])
        kv16_sb = cpool.tile([128, tiles_total * 8], I16, tag="kv16")
        nc.sync.dma_start(kv16_sb[:], kv16_in[:])
        xT_sb = cpool.tile([C, NPC], F16, tag="xT")
        nc.sync.dma_start(xT_sb[:], xT_in[:])

        ident = cpool.tile([128, 128], F16, tag="ident")
        make_identity(nc, ident[:])
        # dma_gather lives in the 'mlp' GPSIMD ucode library; the identity
        # setup above needs the default library, so swap after.
        nc.gpsimd.load_library(library_config.mlp)

        h1T = cpool.tile([C, NPC], F16, tag="h1T")
        qall = cpool.tile([128, BPC * C], F16, tag="qall")
        aggn_all = cpool.tile([128, BPC * 128], F16, tag="aggn_all")

        # --- DRAM scratch ------------------------------------------------
        kv_shard = dram.tile([NPC, 2 * C], F16, tag="kvs", name="kv_shard")
        kvf_kw = dict(addr_space="Shared") if shared_kvf else {}
        kv_full = [dram.tile([NPAD, 2 * C], F16, tag=f"kvf{l}", name=f"kv_full{l}",
                             **kvf_kw) for l in (0, 1)]

        def layer(li, srcT, asig):
            l = li + 1
            kvf = kv_full[li]
            # ---- projections: one matmul per block ----
            for b in range(BPC):
                blk = slice(b * 128, (b + 1) * 128)
                ps = psA.tile([128, 3 * C], F32, tag="proj")
                nc.tensor.matmul(out=ps[:], lhsT=srcT[:, blk], rhs=W[f"Wqkv{l}"][:],
                                 start=True, stop=True)
                nc.vector.tensor_tensor(out=qall[:, blk], in0=ps[:, 0:C],
                                        in1=W[f"bqkv{l}"][:, 0:C], op=OP.add)
                qkv = sb.tile([128, 2 * C], F16, tag="qkv")
                nc.vector.tensor_tensor(out=qkv[:], in0=ps[:, C:3 * C],
                                        in1=W[f"bqkv{l}"][:, C:3 * C], op=OP.add)
                nc.sync.dma_start(kv_shard[blk, :], qkv[:])
            # ---- exchange k/v ----
            nc.gpsimd.collective_compute(
                "AllGather", OP.bypass,
                replica_groups=[list(range(CORES))],
                ins=[kv_shard[:]], outs=[kvf[:]])

            # ---- edge pass A: gather + attention + aggregate ----
            def gather_rows(dst, dst_off, table, col8, ntiles, qn):
                done = 0
                while done < ntiles:
                    k = min(GCHUNK, ntiles - done)
                    nc.gpsimd.dma_gather(
                        out_ap=_v(dst[:], dst_off + done * 256,
                                  [[256, k], [1, 256]]),
                        in_ap=table,
                        idxs_ap=kv16_sb[:, (col8 + done) * 8:(col8 + done + k) * 8],
                        num_idxs=k * 128, num_idxs_reg=k * 128,
                        elem_size=256, queue_num=qn)
                    done += k

            col = 0
            for b in range(BPC):
                T1, T2 = T1_b[b], T2_b[b]
                T = T1 + T2
                qn = b % nqueues
                blk = slice(b * 128, (b + 1) * 128)
                kvg = sb.tile([128, Tmax * 256], F16, tag="kvg")
                if T1:
                    gather_rows(kvg, 0, kvf[0:HALF, :], col, T1, qn)
                if T2:
                    gather_rows(kvg, T1 * 256, kvf[HALF:NPAD, :], col + T1, T2, qn)
                S = sb.tile([128, Tmax * 128], F16, tag="S")
                nc.sync.dma_start(S[:, :T * 128],
                                  S_in[:, col * 128:(col + T) * 128])
                ST = sb.tile([128, Tmax * 128], F16, tag="ST")
                nc.sync.dma_start(ST[:, :T * 128],
                                  ST_in[:, col * 128:(col + T) * 128])
                kq = sb.tile([128, Tmax * 128], F16, tag="kq")
                for c0 in range(0, T, GCHUNK):
                    k = min(GCHUNK, T - c0)
                    qg = psQ.tile([128, GCHUNK * 128], F32, tag="qg")
                    for t in range(c0, c0 + k):
                        nc.tensor.matmul(out=qg[:, (t - c0) * 128:(t - c0 + 1) * 128],
                                         lhsT=ST[:, t * 128:(t + 1) * 128],
                                         rhs=qall[:, blk], start=True, stop=True)
                    nc.vector.tensor_tensor(
                        out=_v(kq[:], c0 * 128, [[128, k], [1, 128]]),
                        in0=_v(kvg[:], c0 * 256, [[256, k], [1, 128]]),
                        in1=_v(qg[:], 0, [[128, k], [1, 128]]),
                        op=OP.mult)
                alpha = sbs.tile([128, Tmax * 4], F32, tag="alpha")
                nc.vector.tensor_reduce(
                    out=alpha[:, :T * 4],
                    in_=_v(kq[:], 0, [[32, T * 4], [1, 32]]),
                    axis=mybir.AxisListType.X, op=OP.add)
                alc = sbs.tile([128, Tmax * 4], F32, tag="alc")
                nc.vector.tensor_scalar_min(alc[:, :T * 4], alpha[:, :T * 4], ACLAMP)
                ex = sbs.tile([128, Tmax * 4], F16, tag="ex")
                nc.scalar.activation(ex[:, :T * 4], alc[:, :T * 4], AF.Exp)
                r = sb.tile([128, Tmax * 132], F16, tag="r")
                nc.vector.tensor_tensor(
                    out=_v(r[:], 0, [[132, T], [32, 4], [1, 32]]),
                    in0=_v(kvg[:], 128, [[256, T], [32, 4], [1, 32]]),
                    in1=_v(ex[:], 0, [[4, T], [1, 4], [0, 32]]),
                    op=OP.mult)
                nc.vector.tensor_copy(
                    out=_v(r[:], 128, [[132, T], [1, 4]]),
                    in_=_v(ex[:], 0, [[4, T], [1, 4]]))
                agg = psA.tile([128, 132], F32, tag="agg")
                for t in range(T):
                    nc.tensor.matmul(out=agg[:],
                                     lhsT=S[:, t * 128:(t + 1) * 128],
                                     rhs=r[:, t * 132:(t + 1) * 132],
                                     start=(t == 0), stop=(t == T - 1))
                rds = sbs.tile([128, 4], F32, tag="rds")
                nc.vector.tensor_scalar_add(rds[:], agg[:, 128:132], EPS)
                rd = sbs.tile([128, 4], F32, tag="rd")
                nc.vector.reciprocal(rd[:], rds[:])
                nc.vector.tensor_tensor(
                    out=_v(aggn_all[:], b * 128, [[32, 4], [1, 32]]),
                    in0=_v(agg[:], 0, [[32, 4], [1, 32]]),
                    in1=_v(rd[:], 0, [[1, 4], [0, 32]]),
                    op=OP.mult)
                col += T
            # ---- edge pass B: gelu + output proj + skip ----
            for b in range(BPC):
                blk = slice(b * 128, (b + 1) * 128)
                anT = psB.tile([128, 128], F16, tag="anT")
                nc.tensor.transpose(out=anT[:], in_=aggn_all[:, blk],
                                    identity=ident[:])
                gT = sbs.tile([128, 128], F16, tag="gT")
                if gelu_mode == "hw":
                    nc.scalar.activation(gT[:], anT[:], AF.Gelu)
                else:
                    # sim-only tanh-approx gelu (CoreSim lacks Gelu/Erf)
                    t1 = sbs.tile([128, 128], F32, tag="gel1")
                    nc.scalar.activation(t1[:], anT[:], AF.Square)
                    nc.vector.tensor_tensor(out=t1[:], in0=t1[:], in1=anT[:], op=OP.mult)
                    nc.vector.tensor_scalar_mul(t1[:], t1[:], 0.044715)
                    nc.vector.tensor_tensor(out=t1[:], in0=t1[:], in1=anT[:], op=OP.add)
                    nc.scalar.activation(t1[:], t1[:], AF.Tanh, scale=0.7978845608028654)
                    nc.vector.tensor_scalar_add(t1[:], t1[:], 1.0)
                    nc.vector.tensor_tensor(out=t1[:], in0=t1[:], in1=anT[:], op=OP.mult)
                    nc.vector.tensor_scalar_mul(gT[:], t1[:], 0.5)
                hps = psB.tile([128, 128], F32, tag="hps")
                nc.tensor.matmul(out=hps[:], lhsT=W[f"Wo{l}"][:], rhs=gT[:],
                                 start=True, stop=True)
                ha = sbs.tile([128, 128], F16, tag="ha")
                nc.scalar.activation(ha[:], hps[:], AF.Identity,
                                     bias=W[f"boaT{l}"][:], scale=asig)
                if l == 1:
                    nc.vector.scalar_tensor_tensor(
                        out=h1T[:, blk], in0=srcT[:, blk], scalar=1.0 - asig,
                        in1=ha[:], op0=OP.mult, op1=OP.add)
                else:
                    hm = sbs.tile([128, 128], F16, tag="hm")
                    nc.vector.scalar_tensor_tensor(
                        out=hm[:], in0=srcT[:, blk], scalar=1.0 - asig,
                        in1=ha[:], op0=OP.mult, op1=OP.add)
                    uvp = psB.tile([2, 128], F32, tag="uvp")
                    nc.tensor.matmul(out=uvp[:], lhsT=W["w12"][:], rhs=hm[:],
                                     start=True, stop=True)
                    uvt = sbs.tile([2, 128], F32, tag="uvt")
                    nc.vector.tensor_copy(uvt[:], uvp[:])
                    nc.sync.dma_start(uv_out[:, blk], uvt[:])

        layer(0, xT_sb[:], asig1)
        layer(1, h1T[:], asig2)

    nc.compile()
    return nc


_CACHE = {}


def _get_program(meta, asig1, asig2, blp, gelu_mode=None, shared_kvf=None,
                 nqueues=None):
    if gelu_mode is None:
        gelu_mode = os.environ.get("HGT_GELU", "hw")
    if shared_kvf is None:
        shared_kvf = os.environ.get("HGT_SHARED_KVF", "1") == "1"
    if nqueues is None:
        nqueues = int(os.environ.get("HGT_NQUEUES", "1"))
    key = (meta["N"], meta["E"], meta["P"], meta["T1_b"], meta["T2_b"],
           asig1, asig2, gelu_mode, shared_kvf, nqueues)
    if key not in _CACHE:
        _CACHE[key] = _build_program(meta, asig1, asig2, gelu_mode, shared_kvf,
                                     nqueues)
    return _CACHE[key]


def make_in_maps(inputs):
    inputs = {k: np.asarray(v) for k, v in inputs.items()}
    H, D = inputs["a1"].shape[0], inputs["a1"].shape[1]
    meta, arrays = _host_prep(inputs["x"].astype(np.float32),
                              inputs["edge_index"],
                              inputs["pos_edge_index"],
                              inputs["neg_edge_index"])
    w = _prep_weights(inputs, H, D)
    in_maps = []
    for c in range(CORES):
        m = dict(xT=arrays["xT"][c], kv16=arrays["kv16"][c],
                 S_hot=arrays["S"][c], ST_hot=arrays["ST"][c])
        for n in ("Wqkv1", "Wqkv2", "bqkv1", "bqkv2", "Wo1", "Wo2",
                  "boaT1", "boaT2", "w12"):
            m[n] = w[n]
        in_maps.append(m)
    return meta, w, in_maps


def assemble(meta, results, inputs, blp):
    uv = np.concatenate([results[c]["uvT_out"] for c in range(CORES)], axis=1)
    u1, u2 = uv[0], uv[1]
    pe, ne = inputs["pos_edge_index"], inputs["neg_edge_index"]
    pos = u1[pe[0]] + u2[pe[1]] + np.float32(blp)
    neg = u1[ne[0]] + u2[ne[1]] + np.float32(blp)
    return pos.astype(np.float32), neg.astype(np.float32)


def kernel(**inputs):
    meta, w, in_maps = make_in_maps(inputs)
    nc = _get_program(meta, w["asig1"], w["asig2"], w["blp"])
    res = bass_utils.run_bass_kernel_spmd(nc, in_maps,
                                          core_ids=list(range(CORES)))
    return assemble(meta, res.results, inputs, w["blp"])


# revision 24
# speedup vs baseline: 5.6710x; 1.3689x over previous
"""HGT link predictor on 8 Trainium2 NeuronCores (Bass/Tile SPMD kernel).

Strategy (hardcoded for nn_HGTLinkPredictor, N=50000 E=800000 P=100000 C=128 H=4 D=32):
 - Shard dst nodes (and their incoming edges) across 8 cores in contiguous
   128-node blocks; edges sorted by dst on host.
 - Features flow in fp16. Node features are kept TRANSPOSED ([C, n]) in SBUF
   so q/k/v projections are a single 384-wide matmul per 128-node block with
   no on-device transposes; relation transforms + attention scale are folded
   into the weights on host.
 - k and v rows are concatenated ([N, 256] fp16); per-edge rows are fetched
   with gpsimd.dma_gather (<=1024 int16 indices per instruction, ~5us each,
   amortized over 8 tiles) instead of per-128-row indirect DMAs. The kv table
   is split in two halves so row indices fit int16; each block's edges are
   reordered low-half-first on the host.
 - q is never round-tripped through DRAM or gathered: q rows stay in SBUF
   ([n, c] per block) and per-edge q is expanded on the tensor engine with
   host-precomputed one-hot selection matrices (qg = ST_t.T @ q_blk).
 - Segment softmax/weighted-sum per 128-node block via the same one-hot
   matrices on the PE; the denominator rides along as 4 extra rhs columns
   and division is deferred to the block epilogue. alpha is clamped at 11
   so exp() fits fp16.
 - The edge phase is split into two passes per layer so the scalar engine
   activation table is not thrashed between Exp and Gelu per block.
 - Epilogue is done transposed (lhsT=Wo trick) so h1^T stays in SBUF for
   layer 2 and the link decode is a [C,2]-stationary matmul per block.
"""

import math
import os
import numpy as np
from contextlib import ExitStack

import concourse.bass as bass
import concourse.tile as tile
from concourse import bacc, mybir
from concourse import bass_utils
from concourse.masks import make_identity
from concourse import library_config

F32 = mybir.dt.float32
F16 = mybir.dt.float16
I16 = mybir.dt.int16
AF = mybir.ActivationFunctionType
OP = mybir.AluOpType

CORES = 8
EPS = 1e-30
ACLAMP = 11.0
GCHUNK = 8          # dma_gather tiles per instruction (1024 idxs)


def _v(ap, off, dims):
    """Custom free-dim view of a 2D [part, width] AP: keep partition dim,
    replace free dims with `dims` ([step, num] pairs), add `off` elements."""
    return bass.AP(ap.tensor, ap.offset + off, [list(ap.ap[0])] + [list(d) for d in dims])


def _wrap16(flat):
    """[M*16] -> [16, M] with element i at [i%16, i//16]."""
    return flat.reshape(-1, 16).T.copy()


# ----------------------------------------------------------------- host prep

def _host_prep(x, edge_index, pos_edge_index, neg_edge_index):
    N, C = x.shape
    E = edge_index.shape[1]
    P = pos_edge_index.shape[1]

    NPC = int(math.ceil(N / (CORES * 128))) * 128   # nodes per core (padded)
    BPC = NPC // 128                                # blocks per core
    NPAD = NPC * CORES
    HALF = NPAD // 2
    assert HALF < 2 ** 15

    src = edge_index[0].astype(np.int64)
    dst = edge_index[1].astype(np.int64)
    order = np.argsort(dst, kind="stable")
    s_src, s_dst = src[order], dst[order]

    core_of = s_dst // NPC
    blk_of = (s_dst % NPC) // 128
    gblk = core_of * BPC + blk_of
    ishigh = (s_src >= HALF).astype(np.int64)

    # reorder within each (core, block): low-half src first
    order2 = np.argsort(gblk * 2 + ishigh, kind="stable")
    s_src, s_dst = s_src[order2], s_dst[order2]
    core_of, blk_of, gblk, ishigh = (core_of[order2], blk_of[order2],
                                     gblk[order2], ishigh[order2])

    # per (core, block, half) counts -> shared tile counts per block index
    cnt = np.zeros((CORES, BPC, 2), dtype=np.int64)
    np.add.at(cnt, (core_of, blk_of, ishigh), 1)
    T1_b = np.ceil(cnt[:, :, 0].max(axis=0) / 128).astype(np.int64)  # [BPC]
    T2_b = np.ceil(cnt[:, :, 1].max(axis=0) / 128).astype(np.int64)
    empty = (T1_b + T2_b) == 0
    T1_b[empty] = 1
    T_b = T1_b + T2_b
    tiles_total = int(T_b.sum())
    tile_start = np.concatenate([[0], np.cumsum(T_b)])[:-1]          # [BPC]

    # rank of each edge within its (core, block, half) group
    ghalf = gblk * 2 + ishigh
    grp_start = np.zeros(CORES * BPC * 2 + 1, dtype=np.int64)
    np.add.at(grp_start, ghalf + 1, 1)
    grp_start = np.cumsum(grp_start)
    pos_in_grp = np.arange(E) - grp_start[ghalf]

    # flat slot within the core's [tiles_total*128] edge array
    flat_pos = (tile_start[blk_of] * 128 + ishigh * T1_b[blk_of] * 128
                + pos_in_grp)

    cap = tiles_total * 128
    kvidx = np.zeros((CORES, cap), dtype=np.int16)
    eslot = np.full((CORES, cap), -1, dtype=np.int64)

    kvidx[core_of, flat_pos] = (s_src - ishigh * HALF).astype(np.int16)
    eslot[core_of, flat_pos] = s_dst % 128

    # one-hot selection matrices, [128, tiles_total*128] fp16
    #   S[p, t*128 + n]  = (eslot[edge t*128+p] == n)
    #   ST[n, t*128 + p] = (eslot[edge t*128+p] == n)
    S = np.zeros((CORES, 128, tiles_total * 128), dtype=np.float16)
    ST = np.zeros((CORES, 128, tiles_total * 128), dtype=np.float16)
    for c in range(CORES):
        i = np.arange(cap)
        valid = eslot[c] >= 0
        iv, sl = i[valid], eslot[c][valid]
        S[c, iv % 128, (iv // 128) * 128 + sl] = 1.0
        ST[c, sl, iv] = 1.0

    kv16 = np.zeros((CORES, 128, tiles_total * 8), dtype=np.int16)
    for c in range(CORES):
        # the SWDGE ucode reads the [16, M] wrap from partition group
        # 2*queue_num(+1); replicate everywhere so any queue works
        kv16[c] = np.tile(_wrap16(kvidx[c]), (8, 1))

    # x shards, transposed: [C, NPC] fp16 (plus residual-prescaled copy)
    xpad = np.zeros((NPAD, C), dtype=np.float32)
    xpad[:N] = x
    xT = np.zeros((CORES, C, NPC), dtype=np.float16)
    for c in range(CORES):
        xT[c] = xpad[c * NPC:(c + 1) * NPC].T.astype(np.float16)

    meta = dict(N=N, C=C, E=E, P=P, NPC=NPC, BPC=BPC, NPAD=NPAD, HALF=HALF,
                T1_b=tuple(int(t) for t in T1_b),
                T2_b=tuple(int(t) for t in T2_b),
                tiles_total=tiles_total)
    arrays = dict(kv16=kv16, S=S, ST=ST, xT=xT)
    return meta, arrays


def _prep_weights(inputs, H, D):
    """Fold relation transforms + attention scale into the linear weights."""
    C = inputs["W1k"].shape[0]
    out = {}
    for l in (1, 2):
        a_rel = np.asarray(inputs[f"a{l}"], np.float64)
        m_rel = np.asarray(inputs[f"m{l}"], np.float64)
        p_rel = np.asarray(inputs[f"p{l}"], np.float64)
        A = np.zeros((C, C)); M = np.zeros((C, C))
        for h in range(H):
            A[h * D:(h + 1) * D, h * D:(h + 1) * D] = a_rel[h]
            M[h * D:(h + 1) * D, h * D:(h + 1) * D] = m_rel[h]
        qscale = np.repeat(p_rel / np.sqrt(D), D)
        Wq = np.asarray(inputs[f"W{l}q"], np.float64) * qscale
        bq = np.asarray(inputs[f"b{l}q"], np.float64) * qscale
        Wk = np.asarray(inputs[f"W{l}k"], np.float64) @ A
        bk = np.asarray(inputs[f"b{l}k"], np.float64) @ A
        Wv = np.asarray(inputs[f"W{l}v"], np.float64) @ M
        bv = np.asarray(inputs[f"b{l}v"], np.float64) @ M
        a_sig = float(1.0 / (1.0 + np.exp(-float(inputs[f"skip{l}"]))))
        Wqkv = np.concatenate([Wq, Wk, Wv], axis=1)        # [C, 384]
        bqkv = np.concatenate([bq, bk, bv])                # [384]
        out[f"Wqkv{l}"] = Wqkv.astype(np.float16)
        out[f"bqkv{l}"] = np.broadcast_to(bqkv.astype(np.float32), (128, 3 * C)).copy()
        out[f"Wo{l}"] = np.asarray(inputs[f"Wo{l}"], np.float16)
        out[f"boaT{l}"] = (a_sig * np.asarray(inputs[f"bo{l}"], np.float64)
                           ).astype(np.float32).reshape(C, 1).copy()
        out[f"asig{l}"] = a_sig
    Wlp = np.asarray(inputs["Wlp"], np.float32)
    out["w12"] = np.stack([Wlp[:C, 0], Wlp[C:, 0]], axis=1).astype(np.float16)  # [C,2]
    out["w12b"] = ((1.0 - out["asig2"]) * np.stack([Wlp[:C, 0], Wlp[C:, 0]], axis=1)
                   ).astype(np.float16)
    out["blp"] = float(np.asarray(inputs["blp"]).reshape(-1)[0])
    return out


# ------------------------------------------------------------------- program

def _build_program(meta, asig1, asig2, gelu_mode="hw", shared_kvf=True,
                   nqueues=1):
    NPC, BPC, NPAD, HALF = meta["NPC"], meta["BPC"], meta["NPAD"], meta["HALF"]
    T1_b, T2_b = meta["T1_b"], meta["T2_b"]
    tiles_total = meta["tiles_total"]
    T_b = [a + b for a, b in zip(T1_b, T2_b)]
    Tmax = max(T_b)
    C = meta["C"]

    nc = bacc.Bacc("TRN2", target_bir_lowering=False, debug=False,
                   num_devices=CORES, num_swdge_queues=nqueues)

    # --- I/O -------------------------------------------------------------
    xT_in = nc.dram_tensor("xT", [C, NPC], F16, kind="ExternalInput").ap()
    xTs_in = nc.dram_tensor("xTs", [C, NPC], F16, kind="ExternalInput").ap()
    kv16_in = nc.dram_tensor("kv16", [128, tiles_total * 8], I16,
                             kind="ExternalInput").ap()
    S_in = nc.dram_tensor("S_hot", [128, tiles_total * 128], F16,
                          kind="ExternalInput").ap()
    ST_in = nc.dram_tensor("ST_hot", [128, tiles_total * 128], F16,
                           kind="ExternalInput").ap()
    w_specs = [("Wqkv1", [C, 3 * C], F16), ("Wqkv2", [C, 3 * C], F16),
               ("bqkv1", [128, 3 * C], F32), ("bqkv2", [128, 3 * C], F32),
               ("Wo1", [C, C], F16), ("Wo2", [C, C], F16),
               ("boaT1", [C, 1], F32), ("boaT2", [C, 1], F32),
               ("w12", [C, 2], F16), ("w12b", [C, 2], F16)]
    w_in = {n: nc.dram_tensor(n, shp, dt, kind="ExternalInput").ap()
            for (n, shp, dt) in w_specs}
    uv_out = nc.dram_tensor("uvT_out", [2, NPC], F32, kind="ExternalOutput").ap()

    with tile.TileContext(nc) as tc, ExitStack() as ctx:
        sb = ctx.enter_context(tc.tile_pool(name="sb", bufs=3))
        sbs = ctx.enter_context(tc.tile_pool(name="sbs", bufs=3))
        cpool = ctx.enter_context(tc.tile_pool(name="const", bufs=1))
        psA = ctx.enter_context(tc.tile_pool(name="psA", bufs=1, space="PSUM"))
        psQ = ctx.enter_context(tc.tile_pool(name="psQ", bufs=1, space="PSUM"))
        psB = ctx.enter_context(tc.tile_pool(name="psB", bufs=1, space="PSUM"))
        dram = ctx.enter_context(tc.tile_pool(name="dr", bufs=1, space="DRAM"))

        # --- constants into SBUF ----------------------------------------
        W = {}
        for (n, shp, dt) in w_specs:
            W[n] = cpool.tile(shp, dt, tag=f"w_{n}", name=f"wt_{n}")
            nc.sync.dma_start(W[n][:], w_in[n][:])
        kv16_sb = cpool.tile([128, tiles_total * 8], I16, tag="kv16")
        nc.sync.dma_start(kv16_sb[:], kv16_in[:])
        xT_sb = cpool.tile([C, NPC], F16, tag="xT")
        nc.sync.dma_start(xT_sb[:], xT_in[:])
        xTs_sb = cpool.tile([C, NPC], F16, tag="xTs")
        nc.sync.dma_start(xTs_sb[:], xTs_in[:])

        ident = cpool.tile([128, 128], F16, tag="ident")
        make_identity(nc, ident[:])
        # dma_gather lives in the 'mlp' GPSIMD ucode library; the identity
        # setup above needs the default library, so swap after.
        nc.gpsimd.load_library(library_config.mlp)

        h1T = cpool.tile([C, NPC], F16, tag="h1T")
        qall = cpool.tile([128, BPC * C], F16, tag="qall")
        aggn_all = cpool.tile([128, BPC * 128], F16, tag="aggn_all")

        # --- DRAM scratch ------------------------------------------------
        kv_shard = dram.tile([NPC, 2 * C], F16, tag="kvs", name="kv_shard")
        kvf_kw = dict(addr_space="Shared") if shared_kvf else {}
        kv_full = [dram.tile([NPAD, 2 * C], F16, tag=f"kvf{l}", name=f"kv_full{l}",
                             **kvf_kw) for l in (0, 1)]

        def layer(li, srcT, asig):
            l = li + 1
            kvf = kv_full[li]
            # ---- projections: one matmul per block ----
            for b in range(BPC):
                blk = slice(b * 128, (b + 1) * 128)
                ps = psA.tile([128, 3 * C], F32, tag="proj")
                nc.tensor.matmul(out=ps[:], lhsT=srcT[:, blk], rhs=W[f"Wqkv{l}"][:],
                                 start=True, stop=True)
                nc.vector.tensor_tensor(out=qall[:, blk], in0=ps[:, 0:C],
                                        in1=W[f"bqkv{l}"][:, 0:C], op=OP.add)
                qkv = sb.tile([128, 2 * C], F16, tag="qkv")
                nc.vector.tensor_tensor(out=qkv[:], in0=ps[:, C:3 * C],
                                        in1=W[f"bqkv{l}"][:, C:3 * C], op=OP.add)
                nc.sync.dma_start(kv_shard[blk, :], qkv[:])
            # ---- exchange k/v ----
            nc.gpsimd.collective_compute(
                "AllGather", OP.bypass,
                replica_groups=[list(range(CORES))],
                ins=[kv_shard[:]], outs=[kvf[:]])

            # ---- edge pass A: gather + attention + aggregate ----
            def gather_rows(dst, dst_off, table, col8, ntiles, qn):
                done = 0
                while done < ntiles:
                    k = min(GCHUNK, ntiles - done)
                    nc.gpsimd.dma_gather(
                        out_ap=_v(dst[:], dst_off + done * 256,
                                  [[256, k], [1, 256]]),
                        in_ap=table,
                        idxs_ap=kv16_sb[:, (col8 + done) * 8:(col8 + done + k) * 8],
                        num_idxs=k * 128, num_idxs_reg=k * 128,
                        elem_size=256, queue_num=qn)
                    done += k

            col = 0
            for b in range(BPC):
                T1, T2 = T1_b[b], T2_b[b]
                T = T1 + T2
                qn = b % nqueues
                blk = slice(b * 128, (b + 1) * 128)
                kvg = sb.tile([128, Tmax * 256], F16, tag="kvg")
                if T1:
                    gather_rows(kvg, 0, kvf[0:HALF, :], col, T1, qn)
                if T2:
                    gather_rows(kvg, T1 * 256, kvf[HALF:NPAD, :], col + T1, T2, qn)
                S = sb.tile([128, Tmax * 128], F16, tag="S")
                nc.sync.dma_start(S[:, :T * 128],
                                  S_in[:, col * 128:(col + T) * 128])
                ST = sb.tile([128, Tmax * 128], F16, tag="ST")
                nc.sync.dma_start(ST[:, :T * 128],
                                  ST_in[:, col * 128:(col + T) * 128])
                kq = sb.tile([128, Tmax * 128], F16, tag="kq")
                for c0 in range(0, T, GCHUNK):
                    k = min(GCHUNK, T - c0)
                    qg = psQ.tile([128, GCHUNK * 128], F32, tag="qg")
                    for t in range(c0, c0 + k):
                        nc.tensor.matmul(out=qg[:, (t - c0) * 128:(t - c0 + 1) * 128],
                                         lhsT=ST[:, t * 128:(t + 1) * 128],
                                         rhs=qall[:, blk], start=True, stop=True)
                    nc.vector.tensor_tensor(
                        out=_v(kq[:], c0 * 128, [[128, k], [1, 128]]),
                        in0=_v(kvg[:], c0 * 256, [[256, k], [1, 128]]),
                        in1=_v(qg[:], 0, [[128, k], [1, 128]]),
                        op=OP.mult)
                alpha = sbs.tile([128, Tmax * 4], F32, tag="alpha")
                nc.vector.tensor_reduce(
                    out=alpha[:, :T * 4],
                    in_=_v(kq[:], 0, [[32, T * 4], [1, 32]]),
                    axis=mybir.AxisListType.X, op=OP.add)
                ex = sbs.tile([128, Tmax * 4], F16, tag="ex")
                nc.scalar.activation(ex[:, :T * 4], alpha[:, :T * 4], AF.Exp)
                r = sb.tile([128, Tmax * 132], F16, tag="r")
                nc.vector.tensor_tensor(
                    out=_v(r[:], 0, [[132, T], [32, 4], [1, 32]]),
                    in0=_v(kvg[:], 128, [[256, T], [32, 4], [1, 32]]),
                    in1=_v(ex[:], 0, [[4, T], [1, 4], [0, 32]]),
                    op=OP.mult)
                nc.scalar.activation(
                    out=_v(r[:], 128, [[132, T], [1, 4]]),
                    in_=_v(ex[:], 0, [[4, T], [1, 4]]), func=AF.Identity)
                agg = psA.tile([128, 132], F32, tag="agg")
                for t in range(T):
                    nc.tensor.matmul(out=agg[:],
                                     lhsT=S[:, t * 128:(t + 1) * 128],
                                     rhs=r[:, t * 132:(t + 1) * 132],
                                     start=(t == 0), stop=(t == T - 1))
                rds = sbs.tile([128, 4], F32, tag="rds")
                nc.vector.tensor_scalar_add(rds[:], agg[:, 128:132], EPS)
                rd = sbs.tile([128, 4], F32, tag="rd")
                nc.vector.reciprocal(rd[:], rds[:])
                nc.vector.tensor_tensor(
                    out=_v(aggn_all[:], b * 128, [[32, 4], [1, 32]]),
                    in0=_v(agg[:], 0, [[32, 4], [1, 32]]),
                    in1=_v(rd[:], 0, [[1, 4], [0, 32]]),
                    op=OP.mult)
                col += T
            # ---- edge pass B: gelu + output proj + skip ----
            for b in range(BPC):
                blk = slice(b * 128, (b + 1) * 128)
                anT = psB.tile([128, 128], F16, tag="anT")
                nc.tensor.transpose(out=anT[:], in_=aggn_all[:, blk],
                                    identity=ident[:])
                gT = sbs.tile([128, 128], F16, tag="gT")
                if gelu_mode == "hw":
                    nc.scalar.activation(gT[:], anT[:], AF.Gelu)
                else:
                    # sim-only tanh-approx gelu (CoreSim lacks Gelu/Erf)
                    t1 = sbs.tile([128, 128], F32, tag="gel1")
                    nc.scalar.activation(t1[:], anT[:], AF.Square)
                    nc.vector.tensor_tensor(out=t1[:], in0=t1[:], in1=anT[:], op=OP.mult)
                    nc.vector.tensor_scalar_mul(t1[:], t1[:], 0.044715)
                    nc.vector.tensor_tensor(out=t1[:], in0=t1[:], in1=anT[:], op=OP.add)
                    nc.scalar.activation(t1[:], t1[:], AF.Tanh, scale=0.7978845608028654)
                    nc.vector.tensor_scalar_add(t1[:], t1[:], 1.0)
                    nc.vector.tensor_tensor(out=t1[:], in0=t1[:], in1=anT[:], op=OP.mult)
                    nc.vector.tensor_scalar_mul(gT[:], t1[:], 0.5)
                hps = psB.tile([128, 128], F32, tag="hps")
                nc.tensor.matmul(out=hps[:], lhsT=W[f"Wo{l}"][:], rhs=gT[:],
                                 start=True, stop=True)
                ha = sbs.tile([128, 128], F16, tag="ha")
                nc.scalar.activation(ha[:], hps[:], AF.Identity,
                                     bias=W[f"boaT{l}"][:], scale=asig)
                if l == 1:
                    nc.vector.tensor_tensor(out=h1T[:, blk], in0=xTs_sb[:, blk],
                                            in1=ha[:], op=OP.add)
                else:
                    # uv = w12.T @ (asig*out+bo) + ((1-asig)*w12).T @ h1
                    uvp = psB.tile([2, 128], F32, tag="uvp")
                    nc.tensor.matmul(out=uvp[:], lhsT=W["w12"][:], rhs=ha[:],
                                     start=True, stop=False)
                    nc.tensor.matmul(out=uvp[:], lhsT=W["w12b"][:],
                                     rhs=srcT[:, blk], start=False, stop=True)
                    uvt = sbs.tile([2, 128], F32, tag="uvt")
                    nc.scalar.activation(uvt[:], uvp[:], AF.Identity)
                    nc.sync.dma_start(uv_out[:, blk], uvt[:])

        layer(0, xT_sb[:], asig1)
        layer(1, h1T[:], asig2)

    nc.compile()
    return nc


_CACHE = {}


def _get_program(meta, asig1, asig2, blp, gelu_mode=None, shared_kvf=None,
                 nqueues=None):
    if gelu_mode is None:
        gelu_mode = os.environ.get("HGT_GELU", "hw")
    if shared_kvf is None:
        shared_kvf = os.environ.get("HGT_SHARED_KVF", "1") == "1"
    if nqueues is None:
        nqueues = int(os.environ.get("HGT_NQUEUES", "1"))
    key = (meta["N"], meta["E"], meta["P"], meta["T1_b"], meta["T2_b"],
           asig1, asig2, gelu_mode, shared_kvf, nqueues)
    if key not in _CACHE:
        _CACHE[key] = _build_program(meta, asig1, asig2, gelu_mode, shared_kvf,
                                     nqueues)
    return _CACHE[key]


def make_in_maps(inputs):
    inputs = {k: np.asarray(v) for k, v in inputs.items()}
    H, D = inputs["a1"].shape[0], inputs["a1"].shape[1]
    meta, arrays = _host_prep(inputs["x"].astype(np.float32),
                              inputs["edge_index"],
                              inputs["pos_edge_index"],
                              inputs["neg_edge_index"])
    w = _prep_weights(inputs, H, D)
    in_maps = []
    for c in range(CORES):
        m = dict(xT=arrays["xT"][c], kv16=arrays["kv16"][c],
                 S_hot=arrays["S"][c], ST_hot=arrays["ST"][c],
                 xTs=((1.0 - w["asig1"]) * arrays["xT"][c].astype(np.float32)
                      ).astype(np.float16))
        for n in ("Wqkv1", "Wqkv2", "bqkv1", "bqkv2", "Wo1", "Wo2",
                  "boaT1", "boaT2", "w12", "w12b"):
            m[n] = w[n]
        in_maps.append(m)
    return meta, w, in_maps


def assemble(meta, results, inputs, blp):
    uv = np.concatenate([results[c]["uvT_out"] for c in range(CORES)], axis=1)
    u1, u2 = uv[0], uv[1]
    pe, ne = inputs["pos_edge_index"], inputs["neg_edge_index"]
    pos = u1[pe[0]] + u2[pe[1]] + np.float32(blp)
    neg = u1[ne[0]] + u2[ne[1]] + np.float32(blp)
    return pos.astype(np.float32), neg.astype(np.float32)


def kernel(**inputs):
    meta, w, in_maps = make_in_maps(inputs)
    nc = _get_program(meta, w["asig1"], w["asig2"], w["blp"])
    res = bass_utils.run_bass_kernel_spmd(nc, in_maps,
                                          core_ids=list(range(CORES)))
    return assemble(meta, res.results, inputs, w["blp"])
